# revision 1
# baseline (speedup 1.0000x reference)
"""CPMAnt attention kernel for 8 TRN2 NeuronCores.

Sharding: 8 cores = 2 batches x 4 head-groups (4 heads each).
Each core computes its batch's QKV projections for its 4 heads, attention
with position bias, and a row-parallel partial of the output projection.
Host sums the 4 partials per batch (Megatron row-parallel reduce done on
host at gather time; no collectives needed).

All matmuls run in bf16 with f32 PSUM accumulation. The kernel works on
transposed operands end to end so no on-device transposes are needed:
  QT[o,s]  = wqT.T @ hqT      (lhsT=wqT[d,o],  rhs=hqT[d,s])
  KT[o,t]  = wkT.T @ hkvT     (lhsT=wkT[d,o],  rhs=hkvT[d,t])
  V [t,o]  = hkvT.T @ wvT     (lhsT=hkvT[d,t], rhs=wvT[d,o])
  ST[t,s]  = KT_h.T @ QT_h    (lhsT=KT chunk,  rhs=QT s-block)
  ET       = exp(ST) * exp(pbT)        (ACT exp, DVE mult)
  OT[o,s] += V_h.T @ ET       (lhsT=V chunk,   rhs=ET chunk)
  Z [*,s] += ones.T @ ET      (broadcast softmax denominator)
  AT       = OT * recip(Z)
  out[s,m] += AT_h.T @ woT    (lhsT=AT chunk,  rhs=woT m-block)
"""

import math
import os

import numpy as np
import ml_dtypes

import concourse.bass as bass
import concourse.bacc as bacc
import concourse.tile as tile
from concourse import mybir
from concourse.bass_utils import run_bass_kernel_spmd

BF16 = ml_dtypes.bfloat16

# Problem shapes (hardcoded per contest contract).
B, LQ, LK = 2, 2048, 2048
DM, H, DH = 2048, 16, 128
P = 128            # partitions
NCORES = 8
HPC = 4            # heads per core
OC = HPC * DH      # 512 output-proj contraction per core
DC = DM // P       # 16 d-chunks
TC = LK // P       # 16 t-chunks
SB = 4             # s-blocks per 2048
NB = LQ // SB      # 512

Q_SCALE = 1.0 / (math.sqrt(DM) * math.sqrt(DH))
KV_SCALE = 1.0 / math.sqrt(DM)
OUT_SCALE = 1.0 / math.sqrt(H * DH)

_PROGRAM = None          # (nc,) cache
_LAST_RESULTS = None     # BassKernelResults from the most recent run


def build_program():
    """Build + compile the single-core Bass program (SPMD across 8 cores)."""
    f32 = mybir.dt.float32
    bf16 = mybir.dt.bfloat16
    nc = bacc.Bacc()

    hq = nc.dram_tensor("hq", [P, DC, LQ], bf16, kind="ExternalInput")
    hkv = nc.dram_tensor("hkv", [P, DC, LK], bf16, kind="ExternalInput")
    wqT = nc.dram_tensor("wqT", [P, DC, OC], bf16, kind="ExternalInput")
    wkT = nc.dram_tensor("wkT", [P, DC, OC], bf16, kind="ExternalInput")
    wvT = nc.dram_tensor("wvT", [P, DC, OC], bf16, kind="ExternalInput")
    woT = nc.dram_tensor("woT", [P, HPC, DM], bf16, kind="ExternalInput")
    pbe = nc.dram_tensor("pbe", [HPC, P, TC, LQ], bf16, kind="ExternalInput")
    out = nc.dram_tensor("out", [P, LQ // P, DM], f32, kind="ExternalOutput")

    Copy = mybir.ActivationFunctionType.Copy
    Exp = mybir.ActivationFunctionType.Exp
    Mult = mybir.AluOpType.mult

    with tile.TileContext(nc) as tc:
        with (
            tc.tile_pool(name="persist", bufs=1) as persist,
            tc.tile_pool(name="qkv", bufs=1) as qkv,
        ):
            ones_sb = persist.tile([P, P], bf16)
            nc.vector.memset(ones_sb, 1.0)
            woT_sb = persist.tile([P, HPC, DM], bf16)
            nc.sync.dma_start(out=woT_sb, in_=woT[:])
            AT = persist.tile([P, HPC, LQ], bf16)

            QT = qkv.tile([P, HPC, LQ], bf16)
            KT = qkv.tile([P, HPC, LK], bf16)
            V = qkv.tile([P, TC, OC], bf16)

            # ---------------- Phase A: QKV projections ----------------
            with (
                tc.tile_pool(name="wA", bufs=1) as wA,
                tc.tile_pool(name="hstream", bufs=3) as hs,
                tc.tile_pool(name="psA", bufs=6, space="PSUM") as psA,
            ):
                wq_sb = wA.tile([P, DC, OC], bf16)
                nc.sync.dma_start(out=wq_sb, in_=wqT[:])
                wk_sb = wA.tile([P, DC, OC], bf16)
                nc.sync.dma_start(out=wk_sb, in_=wkT[:])
                wv_sb = wA.tile([P, DC, OC], bf16)
                nc.sync.dma_start(out=wv_sb, in_=wvT[:])

                for j in range(SB):  # s-blocks of hidden_q -> QT
                    h_sl = hs.tile([P, DC, NB], bf16, tag="h")
                    nc.sync.dma_start(out=h_sl, in_=hq[:, :, j * NB:(j + 1) * NB])
                    for h in range(HPC):
                        ps = psA.tile([P, NB], f32, tag="psA")
                        for d in range(DC):
                            nc.tensor.matmul(
                                ps,
                                lhsT=wq_sb[:, d, h * P:(h + 1) * P],
                                rhs=h_sl[:, d, :],
                                start=(d == 0),
                                stop=(d == DC - 1),
                            )
                        nc.scalar.activation(
                            QT[:, h, j * NB:(j + 1) * NB], ps, Copy, scale=Q_SCALE
                        )

                for j in range(SB):  # t-blocks of hidden_kv -> KT and V
                    h_sl = hs.tile([P, DC, NB], bf16, tag="h")
                    nc.sync.dma_start(out=h_sl, in_=hkv[:, :, j * NB:(j + 1) * NB])
                    for h in range(HPC):
                        ps = psA.tile([P, NB], f32, tag="psA")
                        for d in range(DC):
                            nc.tensor.matmul(
                                ps,
                                lhsT=wk_sb[:, d, h * P:(h + 1) * P],
                                rhs=h_sl[:, d, :],
                                start=(d == 0),
                                stop=(d == DC - 1),
                            )
                        nc.scalar.activation(
                            KT[:, h, j * NB:(j + 1) * NB], ps, Copy, scale=KV_SCALE
                        )
                    for t4 in range(4):
                        ps = psA.tile([P, NB], f32, tag="psA")
                        for d in range(DC):
                            nc.tensor.matmul(
                                ps,
                                lhsT=h_sl[:, d, t4 * P:(t4 + 1) * P],
                                rhs=wv_sb[:, d, :],
                                start=(d == 0),
                                stop=(d == DC - 1),
                            )
                        nc.scalar.activation(
                            V[:, j * 4 + t4, :], ps, Copy, scale=KV_SCALE
                        )

            # ---------------- Phase B: attention per (head, s-block) ----------------
            with (
                tc.tile_pool(name="pb", bufs=2) as pbp,
                tc.tile_pool(name="es", bufs=4) as esp,
                tc.tile_pool(name="E", bufs=2) as Ep,
                tc.tile_pool(name="rz", bufs=2) as rzp,
                tc.tile_pool(name="psS", bufs=4, space="PSUM") as psS,
                tc.tile_pool(name="psO", bufs=2, space="PSUM") as psO,
                tc.tile_pool(name="psZ", bufs=2, space="PSUM") as psZ,
            ):
                for h in range(HPC):
                    for j in range(SB):
                        sl = slice(j * NB, (j + 1) * NB)
                        pb_sl = pbp.tile([P, TC, NB], bf16, tag="pb")
                        nc.sync.dma_start(out=pb_sl, in_=pbe[h, :, :, sl])
                        E_sl = Ep.tile([P, TC, NB], bf16, tag="E")
                        O_ps = psO.tile([P, NB], f32, tag="psO")
                        Z_ps = psZ.tile([P, NB], f32, tag="psZ")

                        # Interleave AV/Z matmuls two chunks behind the
                        # score matmuls so PE stays busy while ACT exps.
                        def av_z(t):
                            nc.tensor.matmul(
                                O_ps,
                                lhsT=V[:, t, h * DH:(h + 1) * DH],
                                rhs=E_sl[:, t, :],
                                start=(t == 0),
                                stop=(t == TC - 1),
                                skip_group_check=True,
                            )
                            nc.tensor.matmul(
                                Z_ps,
                                lhsT=ones_sb,
                                rhs=E_sl[:, t, :],
                                start=(t == 0),
                                stop=(t == TC - 1),
                                skip_group_check=True,
                            )

                        for t in range(TC):
                            S_ps = psS.tile([P, NB], f32, tag="psS")
                            nc.tensor.matmul(
                                S_ps,
                                lhsT=KT[:, h, t * P:(t + 1) * P],
                                rhs=QT[:, h, sl],
                                start=True,
                                stop=True,
                                skip_group_check=True,
                            )
                            eS = esp.tile([P, NB], bf16, tag="es")
                            nc.scalar.activation(eS, S_ps, Exp)
                            nc.vector.tensor_tensor(
                                E_sl[:, t, :], eS, pb_sl[:, t, :], Mult
                            )
                            if t >= 2:
                                av_z(t - 2)
                        av_z(TC - 2)
                        av_z(TC - 1)

                        rz = rzp.tile([P, NB], f32, tag="rz")
                        nc.vector.reciprocal(rz, Z_ps)
                        nc.vector.tensor_tensor(AT[:, h, sl], O_ps, rz, Mult)

            # ---------------- Phase C: output projection (partial) ----------------
            with (
                tc.tile_pool(name="ostage", bufs=4) as osp,
                tc.tile_pool(name="psC", bufs=4, space="PSUM") as psC,
            ):
                for sc in range(LQ // P):
                    for mb in range(DM // NB):
                        ps = psC.tile([P, NB], f32, tag="psC")
                        for oc in range(HPC):
                            nc.tensor.matmul(
                                ps,
                                lhsT=AT[:, oc, sc * P:(sc + 1) * P],
                                rhs=woT_sb[:, oc, mb * NB:(mb + 1) * NB],
                                start=(oc == 0),
                                stop=(oc == HPC - 1),
                            )
                        ostage = osp.tile([P, NB], f32, tag="os")
                        nc.scalar.activation(ostage, ps, Copy, scale=OUT_SCALE)
                        nc.sync.dma_start(
                            out=out[:, sc, mb * NB:(mb + 1) * NB], in_=ostage
                        )

    nc.compile()
    return nc


def _get_program():
    global _PROGRAM
    if _PROGRAM is None:
        _PROGRAM = build_program()
    return _PROGRAM


def make_in_maps(hidden_q, hidden_kv, attention_mask, position_bias, wq, wk, wv, wo):
    """Host-side shard + transpose + bf16 cast for all 8 cores."""
    f32 = np.float32

    def dxp(x):  # [rows, (dc p)] -> [p, dc, rows-free] with d on partitions
        # x: [n, DM]; return [P, DC, n] = x.T chunked
        n = x.shape[0]
        return np.ascontiguousarray(
            x.reshape(n, DC, P).transpose(2, 1, 0)
        )

    # hidden transposes, one per batch (bf16)
    hq_b = [dxp(np.asarray(hidden_q[b], f32)).astype(BF16) for b in range(B)]
    hkv_b = [dxp(np.asarray(hidden_kv[b], f32)).astype(BF16) for b in range(B)]

    mask = np.asarray(attention_mask)
    mask_all_ones = bool(mask.all())

    w_by_hg = []
    for hg in range(HPC):
        rows = slice(hg * OC, (hg + 1) * OC)
        wqT = dxp(np.asarray(wq[rows], f32)).astype(BF16)   # [P, DC, OC]
        wkT = dxp(np.asarray(wk[rows], f32)).astype(BF16)
        wvT = dxp(np.asarray(wv[rows], f32)).astype(BF16)
        # woT[p, oc, m] = wo[m, hg*OC + oc*P + p]
        woT = np.ascontiguousarray(
            np.asarray(wo[:, rows], f32).reshape(DM, HPC, P).transpose(2, 1, 0)
        ).astype(BF16)
        w_by_hg.append((wqT, wkT, wvT, woT))

    in_maps = []
    for core in range(NCORES):
        b, hg = divmod(core, HPC)
        # pbe[h, p, tc, s] = exp(position_bias[hg*4+h, s, tc*P+p])
        pb_sel = np.asarray(position_bias[hg * HPC:(hg + 1) * HPC], f32)
        pbT = pb_sel.reshape(HPC, LQ, TC, P).transpose(0, 3, 2, 1)  # [h,p,tc,s]
        pbe = np.exp(pbT, dtype=f32)
        if not mask_all_ones:
            # mask folded multiplicatively into exp(pb): zeroed keys drop out
            # of both the numerator and the softmax denominator, matching
            # where(mask, score, -inf) + where(mask, probs, 0).
            mT = mask[b].T.reshape(TC, P, LQ).transpose(1, 0, 2)  # [p,tc,s]
            pbe = pbe * mT[None].astype(f32)
        wqT, wkT, wvT, woT = w_by_hg[hg]
        in_maps.append(
            {
                "hq": hq_b[b],
                "hkv": hkv_b[b],
                "wqT": wqT,
                "wkT": wkT,
                "wvT": wvT,
                "woT": woT,
                "pbe": pbe.astype(BF16),
            }
        )
    return in_maps


def gather_output(results):
    """Sum the 4 row-parallel partials per batch; un-permute to [B, LQ, DM]."""
    out = np.zeros((B, LQ, DM), np.float32)
    for core in range(NCORES):
        b = core // HPC
        part = results[core]["out"]  # [P, LQ//P, DM]
        out[b] += part.transpose(1, 0, 2).reshape(LQ, DM)
    return out


def kernel(hidden_q, hidden_kv, attention_mask, position_bias, wq, wk, wv, wo):
    global _LAST_RESULTS
    nc = _get_program()
    in_maps = make_in_maps(
        hidden_q, hidden_kv, attention_mask, position_bias, wq, wk, wv, wo
    )
    trace = os.environ.get("KERNEL_TRACE", "0") == "1"
    res = run_bass_kernel_spmd(
        nc,
        in_maps,
        core_ids=list(range(NCORES)),
        trace=trace,
        trace_cores=[0] if trace else None,
    )
    _LAST_RESULTS = res
    return gather_output(res.results)
